# revision 3
# baseline (speedup 1.0000x reference)
"""GATv2 message-passing model on 8 Trainium2 NeuronCores (Bass/Tile).

Contract: kernel(**inputs) takes the FULL unsharded inputs of
nn_GATv2Model (N=50000 nodes, E=400000 edges, 64 atom features, 16 edge
features, 8 heads x 64 channels) and returns the FULL [N] float32 output.

Sharding: nodes are partitioned contiguously across the 8 cores
(6250/core); edges (plus self loops) are assigned by destination node so
the per-destination softmax and scatter-add stay local. Instead of a halo
exchange of boundary rows, the (cheap) front stage is replicated: an
AllGather of the per-core [atom||agg] feature block lets every core
compute xl = comb @ W_l for ALL nodes locally (node-major fp16 table in
HBM), so the per-edge gather xs = xl[src] is a purely local indirect DMA.

Device pipeline per core (single SPMD program, core identity only via
per-core inputs):
  A  AllGather [atom||agg]^T (feature-major) -> ag_full
  B  BN stats of msg over all N nodes (replicated, no collective)
  C  msg = relu(bn(.)); xl for all nodes / xr for own nodes via two
     K=64 accumulating matmuls (lhsT = feature-major block, so the
     matmul emits node-major rows directly - no transposes)
  D  per 128-dst chunk x F 128-edge tiles: indirect-gather xs=xl[src];
     one-hot S from dst-local ids (is_equal vs iota); xr expanded
     per-edge via S^T matmul; e = leakyrelu(xs + xr_e);
     score = reduce(e * att); ex = exp(score) (scores are O(0.1): no
     max-subtraction needed); numer += S^T @ (xs * ex),
     denom += S^T @ ex accumulated in PSUM; flush out = numer/denom
  E  BN(out) stats -> AllReduce -> normalize+relu -> @W_p1 (PE
     transposes + 4 accumulating matmuls) -> BN stats -> AllReduce ->
     normalize+relu -> dot W_p2 -> y
Host: atom/agg precompute (BLAS + reduceat), dst-sort + chunk padding of
edges, final y assembly (+b_p2). Constant shifts b_msg/b_gat/b_p1 cancel
inside training-mode BN and are dropped; b_p2 is added on host.

The gpsimd dynamic-DMA queue must be drained (nc.gpsimd.drain()) after
the indirect gathers before any collective: leftover indirect-DMA queue
state wedges NRT's collective execution (reproduced + fixed empirically).
"""
import sys
for _p in ("/opt/trn_rl_repo", "/root/.axon_site/_ro/trn_rl_repo"):
    if _p not in sys.path:
        sys.path.append(_p)

import numpy as np
import concourse.bass as bass
import concourse.bacc as bacc
import concourse.tile as tile
import concourse.mybir as mybir
from concourse.masks import make_identity
from concourse.tile_rust import add_dep_helper

F32 = mybir.dt.float32
F16 = mybir.dt.float16
I32 = mybir.dt.int32
AX = mybir.AxisListType
ALU = mybir.AluOpType
ACT = mybir.ActivationFunctionType

HID = 64
H = 8
HC = 512          # H * 64 channels
NEG = 0.2
EPS = 1e-5
P = 128

N_CORES = 8
NPC = 6250        # nodes per core (N = 50000)
N_NODES = N_CORES * NPC
NDC = 49          # dst chunks of 128 per core (49*128 = 6272)
FC = 512          # front-stage node chunk


def _derive_cfg(F):
    cfg = dict(n_cores=N_CORES, Npc=NPC, F=F, FC=FC)
    cfg["N"] = N_NODES
    cfg["ndc"] = NDC
    sizes = []
    r = NPC
    while r > 0:
        sizes.append(min(FC, r))
        r -= min(FC, r)
    cfg["fc_sizes"] = sizes
    cfg["Npc_pad"] = NDC * P
    return cfg


def _build_program(cfg):
    n_cores = cfg["n_cores"]
    Npc = cfg["Npc"]
    N = cfg["N"]
    ndc = cfg["ndc"]
    F = cfg["F"]
    fc_sizes = cfg["fc_sizes"]
    Npc_pad = cfg["Npc_pad"]
    group = [list(range(n_cores))]

    nc = bacc.Bacc(None, target_bir_lowering=False, debug=False,
                   num_devices=n_cores)

    fm_in = nc.dram_tensor("fm_in", [P, Npc], F32, kind="ExternalInput")
    src_idx = nc.dram_tensor("src_idx", [ndc, P, F], I32, kind="ExternalInput")
    dstloc_in = nc.dram_tensor("dstloc", [ndc, P, F], F32, kind="ExternalInput")
    Wmsg = nc.dram_tensor("Wmsg", [HID, HID], F32, kind="ExternalInput")
    Wl_in = nc.dram_tensor("Wl", [2 * HID, HC], F16, kind="ExternalInput")
    Wr_in = nc.dram_tensor("Wr", [2 * HID, HC], F16, kind="ExternalInput")
    att_in = nc.dram_tensor("att_row", [1, HC], F16, kind="ExternalInput")
    gmsg_in = nc.dram_tensor("g_msg", [HID, 1], F32, kind="ExternalInput")
    bemsg_in = nc.dram_tensor("be_msg", [HID, 1], F32, kind="ExternalInput")
    gbn_in = nc.dram_tensor("g_bn", [1, HC], F32, kind="ExternalInput")
    bebn_in = nc.dram_tensor("be_bn", [1, HC], F32, kind="ExternalInput")
    Wp1_in = nc.dram_tensor("Wp1", [P, 4 * HID], F16, kind="ExternalInput")
    gp_in = nc.dram_tensor("g_p", [1, HID], F32, kind="ExternalInput")
    bep_in = nc.dram_tensor("be_p", [1, HID], F32, kind="ExternalInput")
    Wp2_in = nc.dram_tensor("Wp2_row", [1, HID], F32, kind="ExternalInput")

    y_out = nc.dram_tensor("y_raw", [P, ndc], F32, kind="ExternalOutput")

    fm_local = nc.dram_tensor("fm_local", [P, Npc], F32)
    ag_full = nc.dram_tensor("ag_full", [n_cores * P, Npc], F32,
                             addr_space="Shared")
    xl_dram = nc.dram_tensor("xl_dram", [N, HC], F16)
    xr_dram = nc.dram_tensor("xr_dram", [Npc_pad, HC], F16)
    stats_loc = nc.dram_tensor("stats_loc", [1, 2 * HC], F32)
    stats_glob = nc.dram_tensor("stats_glob", [1, 2 * HC], F32,
                                addr_space="Shared")
    hstats_loc = nc.dram_tensor("hstats_loc", [1, 2 * HID], F32)
    hstats_glob = nc.dram_tensor("hstats_glob", [1, 2 * HID], F32,
                                 addr_space="Shared")
    brow_dram = nc.dram_tensor("brow_dram", [2, HC], F16)
    hrow_dram = nc.dram_tensor("hrow_dram", [2, HID], F16)

    nfc = len(fc_sizes)
    nchunks = n_cores * nfc
    inv_n = 1.0 / float(N)

    with tile.TileContext(nc) as tc:
        with tc.tile_pool(name="persist", bufs=1) as pers:
            iota_f = pers.tile([P, P], F32, tag="iota_f")
            iota_i = pers.tile([P, P], I32, tag="iota_i")
            nc.gpsimd.iota(iota_i[:], pattern=[[1, P]], base=0,
                           channel_multiplier=0)
            nc.vector.tensor_copy(iota_f[:], iota_i[:])
            ident = pers.tile([P, P], F16, tag="ident")
            make_identity(nc, ident[:])
            ones16 = pers.tile([P, 1], F16, tag="ones16")
            nc.gpsimd.memset(ones16[:], 1.0)

            wmsg_sb = pers.tile([HID, HID], F32, tag="wmsg")
            nc.sync.dma_start(wmsg_sb[:], Wmsg[:])
            wl_top = pers.tile([HID, HC], F16, tag="wl_top")
            nc.sync.dma_start(wl_top[:], Wl_in[:HID, :])
            wl_bot = pers.tile([HID, HC], F16, tag="wl_bot")
            nc.sync.dma_start(wl_bot[:], Wl_in[HID:, :])
            wr_top = pers.tile([HID, HC], F16, tag="wr_top")
            nc.sync.dma_start(wr_top[:], Wr_in[:HID, :])
            wr_bot = pers.tile([HID, HC], F16, tag="wr_bot")
            nc.sync.dma_start(wr_bot[:], Wr_in[HID:, :])
            att_b = pers.tile([P, HC], F16, tag="att_b")
            nc.sync.dma_start(att_b[:], att_in[:].to_broadcast([P, HC]))
            gmsg_sb = pers.tile([HID, 1], F32, tag="gmsg")
            nc.sync.dma_start(gmsg_sb[:], gmsg_in[:])
            bemsg_sb = pers.tile([HID, 1], F32, tag="bemsg")
            nc.sync.dma_start(bemsg_sb[:], bemsg_in[:])
            gbn_sb = pers.tile([1, HC], F32, tag="gbn")
            nc.sync.dma_start(gbn_sb[:], gbn_in[:])
            bebn_sb = pers.tile([1, HC], F32, tag="bebn")
            nc.sync.dma_start(bebn_sb[:], bebn_in[:])
            wp1_sb = pers.tile([P, 4 * HID], F16, tag="wp1")
            nc.sync.dma_start(wp1_sb[:], Wp1_in[:])
            gp_sb = pers.tile([1, HID], F32, tag="gp")
            nc.sync.dma_start(gp_sb[:], gp_in[:])
            bep_sb = pers.tile([1, HID], F32, tag="bep")
            nc.sync.dma_start(bep_sb[:], bep_in[:])
            w2_b = pers.tile([P, HID], F32, tag="w2b")
            nc.sync.dma_start(w2_b[:], Wp2_in[:].to_broadcast([P, HID]))

            out_acc = pers.tile([P, ndc * HC], F16, tag="out_acc")
            h_acc = pers.tile([P, ndc * HID], F32, tag="h_acc")
            y_acc = pers.tile([P, ndc], F32, tag="y_acc")
            stats_s = pers.tile([HID, nchunks], F32, tag="stats_s")
            stats_q = pers.tile([HID, nchunks], F32, tag="stats_q")
            scale_m = pers.tile([HID, 1], F32, tag="scale_m")
            shift_m = pers.tile([HID, 1], F32, tag="shift_m")
            scale_b = pers.tile([P, HC], F16, tag="scale_b")
            shift_b = pers.tile([P, HC], F16, tag="shift_b")
            scale2_b = pers.tile([P, HID], F16, tag="scale2_b")
            shift2_b = pers.tile([P, HID], F16, tag="shift2_b")

            # ---- A: allgather [atom||agg]^T ----
            nc.sync.dma_start(fm_local[:], fm_in[:])
            nc.gpsimd.collective_compute(
                "AllGather", ALU.bypass, replica_groups=group,
                ins=[fm_local[:]], outs=[ag_full[:]])

            def fchunks():
                for cb in range(n_cores):
                    off = 0
                    for fs in fc_sizes:
                        yield cb, cb * Npc + off, fs
                        off += fs

            # ---- B: msg BN stats over all nodes ----
            with (
                tc.tile_pool(name="fB", bufs=3) as fB,
                tc.tile_pool(name="fBp", bufs=2, space="PSUM") as fBp,
            ):
                for t, (cb, goff, fs) in enumerate(fchunks()):
                    coff = goff - cb * Npc
                    at_t = fB.tile([HID, fs], F32, tag="at_t")
                    nc.sync.dma_start(
                        at_t[:], ag_full[cb * P:cb * P + HID, coff:coff + fs])
                    agg_t = fB.tile([HID, fs], F32, tag="agg_t")
                    nc.sync.dma_start(
                        agg_t[:], ag_full[cb * P + HID:(cb + 1) * P,
                                          coff:coff + fs])
                    s = fB.tile([HID, fs], F32, tag="s")
                    nc.vector.tensor_add(s[:], at_t[:], agg_t[:])
                    m0 = fBp.tile([HID, fs], F32, space="PSUM")
                    nc.tensor.matmul(m0[:], lhsT=wmsg_sb[:], rhs=s[:],
                                     start=True, stop=True)
                    nc.vector.reduce_sum(stats_s[:, t:t + 1], m0[:], axis=AX.X)
                    scr = fB.tile([HID, fs], F32, tag="scr")
                    nc.scalar.activation(out=scr[:], in_=m0[:],
                                         func=ACT.Square,
                                         accum_out=stats_q[:, t:t + 1])

            with tc.tile_pool(name="mstat", bufs=1) as ms:
                s_tot = ms.tile([HID, 1], F32, tag="s_tot")
                q_tot = ms.tile([HID, 1], F32, tag="q_tot")
                nc.vector.reduce_sum(s_tot[:], stats_s[:], axis=AX.X)
                nc.vector.reduce_sum(q_tot[:], stats_q[:], axis=AX.X)
                nc.vector.tensor_scalar_mul(s_tot[:], s_tot[:], inv_n)
                nc.vector.tensor_scalar_mul(q_tot[:], q_tot[:], inv_n)
                m2 = ms.tile([HID, 1], F32, tag="m2")
                nc.vector.tensor_tensor(out=m2[:], in0=s_tot[:], in1=s_tot[:],
                                        op=ALU.mult)
                nc.vector.tensor_tensor(out=q_tot[:], in0=q_tot[:], in1=m2[:],
                                        op=ALU.subtract)
                nc.vector.tensor_scalar_add(q_tot[:], q_tot[:], EPS)
                nc.scalar.sqrt(q_tot[:], q_tot[:])
                inv_s = ms.tile([HID, 1], F32, tag="inv_s")
                nc.vector.reciprocal(inv_s[:], q_tot[:])
                nc.vector.tensor_tensor(out=scale_m[:], in0=gmsg_sb[:],
                                        in1=inv_s[:], op=ALU.mult)
                nc.vector.tensor_tensor(out=m2[:], in0=s_tot[:],
                                        in1=scale_m[:], op=ALU.mult)
                nc.vector.tensor_tensor(out=shift_m[:], in0=bemsg_sb[:],
                                        in1=m2[:], op=ALU.subtract)

            # ---- C: msg/agg -> xl (all nodes), xr (own nodes) ----
            def msg_agg_chunk(pool, ppool, src_top, src_bot, coff, fs):
                at_t = pool.tile([HID, fs], F32, tag="at_tC")
                nc.sync.dma_start(at_t[:], src_top[:, coff:coff + fs])
                agg_t = pool.tile([HID, fs], F16, tag="agg_tC")
                nc.gpsimd.dma_start(agg_t[:], src_bot[:, coff:coff + fs])
                s = pool.tile([HID, fs], F32, tag="sC")
                nc.vector.tensor_add(s[:], at_t[:], agg_t[:])
                m0 = ppool.tile([HID, fs], F32, space="PSUM")
                nc.tensor.matmul(m0[:], lhsT=wmsg_sb[:], rhs=s[:],
                                 start=True, stop=True)
                msg_t = pool.tile([HID, fs], F16, tag="msgC")
                nc.scalar.activation(out=msg_t[:], in_=m0[:], func=ACT.Relu,
                                     bias=shift_m[:, :1], scale=scale_m[:, :1])
                return msg_t, agg_t

            def emit_xw(pool, ppool, msg_t, agg_t, w_top, w_bot, dram, row0,
                        fs, tagp):
                nsb = -(-fs // P)
                xsb = pool.tile([P, nsb * HC], F16, tag=tagp + "_sb")
                for sb in range(nsb):
                    m = min(P, fs - sb * P)
                    xp = ppool.tile([P, HC], F32, space="PSUM", tag="xw_ps")
                    nc.tensor.matmul(xp[:m, :],
                                     lhsT=msg_t[:, sb * P:sb * P + m],
                                     rhs=w_top[:], start=True, stop=False)
                    nc.tensor.matmul(xp[:m, :],
                                     lhsT=agg_t[:, sb * P:sb * P + m],
                                     rhs=w_bot[:], start=False, stop=True)
                    nc.vector.tensor_copy(xsb[:m, sb * HC:(sb + 1) * HC],
                                          xp[:m, :])
                if fs == nsb * P:
                    nc.sync.dma_start(
                        dram[row0:row0 + fs, :].rearrange("(s p) f -> p s f",
                                                          p=P),
                        xsb[:].rearrange("p (s f) -> p s f", f=HC))
                else:
                    assert nsb == 1
                    nc.sync.dma_start(dram[row0:row0 + fs, :], xsb[:fs, :HC])

            with (
                tc.tile_pool(name="fC", bufs=3) as fC,
                tc.tile_pool(name="fCp", bufs=2, space="PSUM") as fCp,
                tc.tile_pool(name="fCx", bufs=4, space="PSUM") as fCx,
            ):
                for cb, goff, fs in fchunks():
                    coff = goff - cb * Npc
                    msg_t, agg_t = msg_agg_chunk(
                        fC, fCp, ag_full[cb * P:cb * P + HID, :],
                        ag_full[cb * P + HID:(cb + 1) * P, :], coff, fs)
                    emit_xw(fC, fCx, msg_t, agg_t, wl_top, wl_bot, xl_dram,
                            goff, fs, "xl")
                off = 0
                for fs in fc_sizes:
                    msg_t, agg_t = msg_agg_chunk(
                        fC, fCp, fm_in[:HID, :], fm_in[HID:, :], off, fs)
                    emit_xw(fC, fCx, msg_t, agg_t, wr_top, wr_bot, xr_dram,
                            off, fs, "xr")
                    off += fs
                if Npc_pad > Npc:
                    zt = fC.tile([P, HC], F16, tag="zpad")
                    nc.gpsimd.memset(zt[:], 0.0)
                    nc.sync.dma_start(xr_dram[Npc:Npc_pad, :],
                                      zt[:Npc_pad - Npc, :])

            # ---- D: edge stage ----
            indirect_insts = []
            with (
                tc.tile_pool(name="eD", bufs=4) as eD,
                tc.tile_pool(name="eDc", bufs=2) as eDc,
                tc.tile_pool(name="pNum", bufs=2, space="PSUM") as pNum,
                tc.tile_pool(name="pDen", bufs=2, space="PSUM") as pDen,
                tc.tile_pool(name="pTr", bufs=2, space="PSUM") as pTr,
                tc.tile_pool(name="pXre", bufs=2, space="PSUM") as pXre,
            ):
                for i in range(ndc):
                    xr_ch = eDc.tile([P, HC], F16, tag="xr_ch")
                    nc.sync.dma_start(xr_ch[:], xr_dram[i * P:(i + 1) * P, :])
                    sidx = eDc.tile([P, F], I32, tag="sidx")
                    nc.sync.dma_start(sidx[:], src_idx[i])
                    dloc = eDc.tile([P, F], F32, tag="dloc")
                    nc.sync.dma_start(dloc[:], dstloc_in[i])
                    numer = pNum.tile([P, HC], F32, space="PSUM")
                    denom = pDen.tile([P, H], F32, space="PSUM")
                    for f in range(F):
                        xs = eD.tile([P, HC], F16, tag="xs")
                        gi = nc.gpsimd.indirect_dma_start(
                            out=xs[:], out_offset=None, in_=xl_dram[:],
                            in_offset=bass.IndirectOffsetOnAxis(
                                ap=sidx[:, f:f + 1], axis=0))
                        indirect_insts.append(gi)
                        S = eD.tile([P, P], F16, tag="S")
                        nc.vector.tensor_tensor(
                            out=S[:], in0=dloc[:, f:f + 1].to_broadcast([P, P]),
                            in1=iota_f[:], op=ALU.is_equal)
                        stp = pTr.tile([P, P], F16, space="PSUM")
                        nc.tensor.transpose(out=stp[:], in_=S[:],
                                            identity=ident[:])
                        st = eD.tile([P, P], F16, tag="st")
                        nc.vector.tensor_copy(st[:], stp[:])
                        xre = pXre.tile([P, HC], F32, space="PSUM")
                        nc.tensor.matmul(xre[:], lhsT=st[:], rhs=xr_ch[:],
                                         start=True, stop=True)
                        e = eD.tile([P, HC], F16, tag="e")
                        nc.vector.tensor_add(e[:], xs[:], xre[:])
                        e2 = eD.tile([P, HC], F16, tag="e2")
                        nc.vector.tensor_scalar_mul(e2[:], e[:], NEG)
                        nc.vector.tensor_tensor(out=e[:], in0=e[:], in1=e2[:],
                                                op=ALU.max)
                        nc.vector.tensor_tensor(out=e2[:], in0=e[:],
                                                in1=att_b[:], op=ALU.mult)
                        score = eD.tile([P, H], F32, tag="score")
                        nc.vector.reduce_sum(
                            score[:], e2[:].rearrange("p (h c) -> p h c", h=H),
                            axis=AX.X)
                        ex = eD.tile([P, H], F16, tag="ex")
                        nc.scalar.activation(out=ex[:], in_=score[:],
                                             func=ACT.Exp)
                        contrib = eD.tile([P, HC], F16, tag="contrib")
                        nc.vector.tensor_tensor(
                            out=contrib[:].rearrange("p (h c) -> p h c", h=H),
                            in0=xs[:].rearrange("p (h c) -> p h c", h=H),
                            in1=ex[:].unsqueeze(-1).to_broadcast([P, H, 64]),
                            op=ALU.mult)
                        nc.tensor.matmul(numer[:], lhsT=S[:], rhs=contrib[:],
                                         start=(f == 0), stop=(f == F - 1))
                        nc.tensor.matmul(denom[:], lhsT=S[:], rhs=ex[:],
                                         start=(f == 0), stop=(f == F - 1))
                    rden = eDc.tile([P, H], F32, tag="rden")
                    nc.vector.tensor_scalar_max(rden[:], denom[:], 1e-30)
                    nc.vector.reciprocal(rden[:], rden[:])
                    nc.vector.tensor_tensor(
                        out=out_acc[:, i * HC:(i + 1) * HC].rearrange(
                            "p (h c) -> p h c", h=H),
                        in0=numer[:].rearrange("p (h c) -> p h c", h=H),
                        in1=rden[:].unsqueeze(-1).to_broadcast([P, H, 64]),
                        op=ALU.mult)

            # the dynamic-DMA queue must drain before any later collective
            drain_inst = nc.gpsimd.drain()
            for gi in indirect_insts:
                add_dep_helper(drain_inst.ins, gi.ins, sync=False,
                               reason="drain after gathers")

            # ---- E1: out BN stats + AllReduce ----
            with (
                tc.tile_pool(name="e1", bufs=3) as e1,
                tc.tile_pool(name="e1p", bufs=1, space="PSUM") as e1p,
            ):
                sum_ps = e1p.tile([1, HC], F32, space="PSUM", tag="sum_ps")
                q_ps = e1p.tile([1, HC], F32, space="PSUM", tag="q_ps")
                for i in range(ndc):
                    osl = out_acc[:, i * HC:(i + 1) * HC]
                    sq = e1.tile([P, HC], F16, tag="sq")
                    nc.vector.tensor_tensor(out=sq[:], in0=osl, in1=osl,
                                            op=ALU.mult)
                    nc.tensor.matmul(sum_ps[:], lhsT=ones16[:], rhs=osl,
                                     start=(i == 0), stop=(i == ndc - 1))
                    nc.tensor.matmul(q_ps[:], lhsT=ones16[:], rhs=sq[:],
                                     start=(i == 0), stop=(i == ndc - 1))
                srow = e1.tile([1, 2 * HC], F32, tag="srow")
                nc.vector.tensor_copy(srow[:, :HC], sum_ps[:])
                nc.vector.tensor_copy(srow[:, HC:], q_ps[:])
                nc.sync.dma_start(stats_loc[:], srow[:])
                ar1 = nc.gpsimd.collective_compute(
                    "AllReduce", ALU.add, replica_groups=group,
                    ins=[stats_loc[:]], outs=[stats_glob[:]])
                add_dep_helper(ar1.ins, drain_inst.ins, sync=False,
                               reason="collective after drain")
                grow = e1.tile([1, 2 * HC], F32, tag="grow")
                nc.sync.dma_start(grow[:], stats_glob[:])
                mean = e1.tile([1, HC], F32, tag="mean")
                nc.vector.tensor_scalar_mul(mean[:], grow[:, :HC], inv_n)
                var = e1.tile([1, HC], F32, tag="var")
                nc.vector.tensor_scalar_mul(var[:], grow[:, HC:], inv_n)
                m2r = e1.tile([1, HC], F32, tag="m2r")
                nc.vector.tensor_tensor(out=m2r[:], in0=mean[:], in1=mean[:],
                                        op=ALU.mult)
                nc.vector.tensor_tensor(out=var[:], in0=var[:], in1=m2r[:],
                                        op=ALU.subtract)
                nc.vector.tensor_scalar_add(var[:], var[:], EPS)
                nc.scalar.sqrt(var[:], var[:])
                inv_r = e1.tile([1, HC], F32, tag="inv_r")
                nc.vector.reciprocal(inv_r[:], var[:])
                sc_row = e1.tile([1, HC], F16, tag="sc_row")
                sh_rowf = e1.tile([1, HC], F32, tag="sh_rowf")
                nc.vector.tensor_tensor(out=inv_r[:], in0=gbn_sb[:],
                                        in1=inv_r[:], op=ALU.mult)
                nc.vector.tensor_copy(sc_row[:], inv_r[:])
                nc.vector.tensor_tensor(out=sh_rowf[:], in0=mean[:],
                                        in1=inv_r[:], op=ALU.mult)
                nc.vector.tensor_tensor(out=sh_rowf[:], in0=bebn_sb[:],
                                        in1=sh_rowf[:], op=ALU.subtract)
                sh_row = e1.tile([1, HC], F16, tag="sh_row")
                nc.vector.tensor_copy(sh_row[:], sh_rowf[:])
                nc.sync.dma_start(brow_dram[0:1, :], sc_row[:])
                nc.sync.dma_start(brow_dram[1:2, :], sh_row[:])
                nc.sync.dma_start(scale_b[:],
                                  brow_dram[0:1, :].to_broadcast([P, HC]))
                nc.sync.dma_start(shift_b[:],
                                  brow_dram[1:2, :].to_broadcast([P, HC]))

            # ---- E2: h = relu(bn(out)) @ W_p1, h BN stats + AllReduce ----
            with (
                tc.tile_pool(name="e2", bufs=3) as e2,
                tc.tile_pool(name="e2p", bufs=2, space="PSUM") as e2p,
                tc.tile_pool(name="e2t", bufs=2, space="PSUM") as e2t,
                tc.tile_pool(name="e2s", bufs=1, space="PSUM") as e2s,
            ):
                hs_ps = e2s.tile([1, HID], F32, space="PSUM", tag="hs_ps")
                hq_ps = e2s.tile([1, HID], F32, space="PSUM", tag="hq_ps")
                for i in range(ndc):
                    osl = out_acc[:, i * HC:(i + 1) * HC]
                    on = e2.tile([P, HC], F16, tag="on")
                    nc.vector.tensor_tensor(out=on[:], in0=osl, in1=scale_b[:],
                                            op=ALU.mult)
                    nc.vector.tensor_tensor(out=on[:], in0=on[:],
                                            in1=shift_b[:], op=ALU.add)
                    nc.scalar.activation(out=on[:], in_=on[:], func=ACT.Relu)
                    hp = e2p.tile([P, HID], F32, space="PSUM", tag="hp")
                    for kb in range(4):
                        otp = e2t.tile([P, P], F16, space="PSUM", tag="otp")
                        nc.tensor.transpose(out=otp[:],
                                            in_=on[:, kb * P:(kb + 1) * P],
                                            identity=ident[:])
                        ot = e2.tile([P, P], F16, tag="ot")
                        nc.vector.tensor_copy(ot[:], otp[:])
                        nc.tensor.matmul(hp[:], lhsT=ot[:],
                                         rhs=wp1_sb[:, kb * HID:(kb + 1) * HID],
                                         start=(kb == 0), stop=(kb == 3))
                    hsl = h_acc[:, i * HID:(i + 1) * HID]
                    nc.vector.tensor_copy(hsl, hp[:])
                    hsq = e2.tile([P, HID], F16, tag="hsq")
                    nc.scalar.square(hsq[:], hp[:])
                    hcp = e2.tile([P, HID], F16, tag="hcp")
                    nc.vector.tensor_copy(hcp[:], hp[:])
                    nc.tensor.matmul(hs_ps[:], lhsT=ones16[:], rhs=hcp[:],
                                     start=(i == 0), stop=(i == ndc - 1))
                    nc.tensor.matmul(hq_ps[:], lhsT=ones16[:], rhs=hsq[:],
                                     start=(i == 0), stop=(i == ndc - 1))
                hrow = e2.tile([1, 2 * HID], F32, tag="hrow")
                nc.vector.tensor_copy(hrow[:, :HID], hs_ps[:])
                nc.vector.tensor_copy(hrow[:, HID:], hq_ps[:])
                nc.sync.dma_start(hstats_loc[:], hrow[:])
                drain2 = nc.gpsimd.drain()
                ar2 = nc.gpsimd.collective_compute(
                    "AllReduce", ALU.add, replica_groups=group,
                    ins=[hstats_loc[:]], outs=[hstats_glob[:]])
                add_dep_helper(ar2.ins, drain2.ins, sync=False,
                               reason="collective after drain")
                hg = e2.tile([1, 2 * HID], F32, tag="hg")
                nc.sync.dma_start(hg[:], hstats_glob[:])
                hmean = e2.tile([1, HID], F32, tag="hmean")
                nc.vector.tensor_scalar_mul(hmean[:], hg[:, :HID], inv_n)
                hvar = e2.tile([1, HID], F32, tag="hvar")
                nc.vector.tensor_scalar_mul(hvar[:], hg[:, HID:], inv_n)
                hm2 = e2.tile([1, HID], F32, tag="hm2")
                nc.vector.tensor_tensor(out=hm2[:], in0=hmean[:], in1=hmean[:],
                                        op=ALU.mult)
                nc.vector.tensor_tensor(out=hvar[:], in0=hvar[:], in1=hm2[:],
                                        op=ALU.subtract)
                nc.vector.tensor_scalar_add(hvar[:], hvar[:], EPS)
                nc.scalar.sqrt(hvar[:], hvar[:])
                hinv = e2.tile([1, HID], F32, tag="hinv")
                nc.vector.reciprocal(hinv[:], hvar[:])
                nc.vector.tensor_tensor(out=hinv[:], in0=gp_sb[:], in1=hinv[:],
                                        op=ALU.mult)
                hsc = e2.tile([1, HID], F16, tag="hsc")
                nc.vector.tensor_copy(hsc[:], hinv[:])
                hshf = e2.tile([1, HID], F32, tag="hshf")
                nc.vector.tensor_tensor(out=hshf[:], in0=hmean[:], in1=hinv[:],
                                        op=ALU.mult)
                nc.vector.tensor_tensor(out=hshf[:], in0=bep_sb[:],
                                        in1=hshf[:], op=ALU.subtract)
                hsh = e2.tile([1, HID], F16, tag="hsh")
                nc.vector.tensor_copy(hsh[:], hshf[:])
                nc.sync.dma_start(hrow_dram[0:1, :], hsc[:])
                nc.sync.dma_start(hrow_dram[1:2, :], hsh[:])
                nc.sync.dma_start(scale2_b[:],
                                  hrow_dram[0:1, :].to_broadcast([P, HID]))
                nc.sync.dma_start(shift2_b[:],
                                  hrow_dram[1:2, :].to_broadcast([P, HID]))

            # ---- E3: y ----
            with tc.tile_pool(name="e3", bufs=3) as e3:
                for i in range(ndc):
                    hsl = h_acc[:, i * HID:(i + 1) * HID]
                    hn = e3.tile([P, HID], F32, tag="hn")
                    nc.vector.tensor_tensor(out=hn[:], in0=hsl,
                                            in1=scale2_b[:], op=ALU.mult)
                    nc.vector.tensor_tensor(out=hn[:], in0=hn[:],
                                            in1=shift2_b[:], op=ALU.add)
                    nc.scalar.activation(out=hn[:], in_=hn[:], func=ACT.Relu)
                    yscr = e3.tile([P, HID], F32, tag="yscr")
                    nc.vector.tensor_tensor(out=yscr[:], in0=hn[:],
                                            in1=w2_b[:], op=ALU.mult)
                    nc.vector.reduce_sum(y_acc[:, i:i + 1], yscr[:],
                                         axis=AX.X)
                nc.sync.dma_start(y_out[:], y_acc[:])

    nc.compile()
    return nc


def _host_prep(inputs, cfg):
    n_cores, Npc, N, ndc, F = (cfg["n_cores"], cfg["Npc"], cfg["N"],
                               cfg["ndc"], cfg["F"])
    x = np.asarray(inputs["x"], np.float32)
    ei = np.asarray(inputs["edge_index"])
    ea = np.asarray(inputs["edge_attr"], np.float32)

    atom = x @ np.asarray(inputs["W_ap"], np.float32) + \
        np.asarray(inputs["b_ap"], np.float32)

    col = ei[1].astype(np.int64)
    cnt = np.bincount(col, minlength=N).astype(np.float32)
    oc = np.argsort(col, kind="stable")
    cs = col[oc]
    bnd = np.flatnonzero(np.r_[True, cs[1:] != cs[:-1]])
    segsum = np.add.reduceat(ea[oc], bnd, axis=0)
    agg = np.zeros((N, HID), np.float32)
    nodes_w = cs[bnd]
    mean16 = segsum / cnt[nodes_w, None]
    agg[nodes_w] = mean16 @ np.asarray(inputs["W_ep"], np.float32) + \
        np.asarray(inputs["b_ep"], np.float32)

    row = ei[0].astype(np.int64)
    ar = np.arange(N, dtype=np.int64)
    src_all = np.concatenate([row, ar])
    dst_all = np.concatenate([col, ar])
    order = np.argsort(dst_all, kind="stable")
    src_s = src_all[order].astype(np.int32)
    dst_s = dst_all[order]
    core = dst_s // Npc
    local = dst_s - core * Npc
    cic = local // P
    gch = core * ndc + cic
    counts = np.bincount(gch, minlength=n_cores * ndc)
    assert int(np.ceil(counts.max() / P)) <= F
    starts = np.zeros(n_cores * ndc + 1, np.int64)
    np.cumsum(counts, out=starts[1:])
    rank = np.arange(len(dst_s)) - starts[gch]
    slot = gch * (F * P) + rank
    nslots = n_cores * ndc * F * P
    src_pad = np.zeros(nslots, np.int32)
    src_pad[slot] = src_s
    dstloc = np.full(nslots, -1.0, np.float32)
    dstloc[slot] = (local - cic * P).astype(np.float32)
    src_pad = src_pad.reshape(n_cores, ndc, F, P).transpose(0, 1, 3, 2).copy()
    dstloc = dstloc.reshape(n_cores, ndc, F, P).transpose(0, 1, 3, 2).copy()

    f32 = lambda k: np.asarray(inputs[k], np.float32)
    f16 = lambda a: np.asarray(a, np.float16)
    Wp1 = f32("W_p1")
    Wp1_packed = Wp1.reshape(4, P, HID).transpose(1, 0, 2).reshape(P, 4 * HID)

    shared = dict(
        Wmsg=f32("W_msg"),
        Wl=f16(f32("W_l")), Wr=f16(f32("W_r")),
        att_row=f16(f32("att").reshape(1, HC)),
        g_msg=f32("g_msg").reshape(HID, 1),
        be_msg=f32("be_msg").reshape(HID, 1),
        g_bn=f32("g_bn").reshape(1, HC), be_bn=f32("be_bn").reshape(1, HC),
        Wp1=f16(Wp1_packed),
        g_p=f32("g_p").reshape(1, HID), be_p=f32("be_p").reshape(1, HID),
        Wp2_row=f32("W_p2").reshape(1, HID),
    )
    in_maps = []
    for k in range(n_cores):
        fm = np.empty((P, Npc), np.float32)
        fm[:HID] = atom[k * Npc:(k + 1) * Npc].T
        fm[HID:] = agg[k * Npc:(k + 1) * Npc].T
        m = dict(shared)
        m["fm_in"] = fm
        m["src_idx"] = src_pad[k]
        m["dstloc"] = dstloc[k]
        in_maps.append(m)
    return in_maps


def _compute_F(edge_index):
    col = np.asarray(edge_index)[1].astype(np.int64)
    dst_all = np.concatenate([col, np.arange(N_NODES, dtype=np.int64)])
    core = dst_all // NPC
    local = dst_all - core * NPC
    gch = core * NDC + local // P
    counts = np.bincount(gch, minlength=N_CORES * NDC)
    return int(np.ceil(counts.max() / P))


_CACHE = {}


def _get_runner(F):
    """Build program + a reusable jitted SPMD callable for edge-budget F."""
    if F in _CACHE:
        return _CACHE[F]
    import jax
    from jax.sharding import Mesh, PartitionSpec
    from jax.experimental.shard_map import shard_map
    from concourse.bass2jax import (_bass_exec_p, install_neuronx_cc_hook,
                                    partition_id_tensor)

    cfg = _derive_cfg(F)
    nc = _build_program(cfg)
    install_neuronx_cc_hook()
    partition_name = (nc.partition_id_tensor.name
                      if nc.partition_id_tensor else None)
    in_names, out_names, out_avals, zero_shapes = [], [], [], []
    for alloc in nc.m.functions[0].allocations:
        if not isinstance(alloc, mybir.MemoryLocationSet):
            continue
        name = alloc.memorylocations[0].name
        if alloc.kind == "ExternalInput":
            if name != partition_name:
                in_names.append(name)
        elif alloc.kind == "ExternalOutput":
            shape = tuple(alloc.tensor_shape)
            dtype = mybir.dt.np(alloc.dtype)
            out_names.append(name)
            out_avals.append(jax.core.ShapedArray(shape, dtype))
            zero_shapes.append((shape, dtype))
    n_params = len(in_names)
    n_outs = len(out_avals)
    all_names = in_names + out_names + (
        [partition_name] if partition_name else [])
    donate = tuple(range(n_params, n_params + n_outs))

    def _body(*args):
        operands = list(args)
        if partition_name is not None:
            operands.append(partition_id_tensor())
        outs = _bass_exec_p.bind(
            *operands, out_avals=tuple(out_avals), in_names=tuple(all_names),
            out_names=tuple(out_names), lowering_input_output_aliases=(),
            sim_require_finite=True, sim_require_nnan=True, nc=nc)
        return tuple(outs)

    devices = jax.devices()[:N_CORES]
    mesh = Mesh(np.asarray(devices), ("core",))
    in_specs = (PartitionSpec("core"),) * (n_params + n_outs)
    out_specs = (PartitionSpec("core"),) * n_outs
    sharded = jax.jit(shard_map(_body, mesh=mesh, in_specs=in_specs,
                                out_specs=out_specs, check_rep=False),
                      donate_argnums=donate, keep_unused=True)

    def run(in_maps):
        per_core = [[np.asarray(m[nm]) for nm in in_names] for m in in_maps]
        concat_in = [
            np.concatenate([per_core[c][i] for c in range(N_CORES)], axis=0)
            for i in range(n_params)]
        concat_zeros = [np.zeros((N_CORES * s[0], *s[1:]), d)
                        for (s, d) in zero_shapes]
        out_arrs = sharded(*concat_in, *concat_zeros)
        outs = [np.asarray(a) for a in out_arrs]
        return [
            {out_names[i]: outs[i].reshape(N_CORES, *out_avals[i].shape)[c]
             for i in range(n_outs)}
            for c in range(N_CORES)]

    _CACHE[F] = (cfg, run)
    return _CACHE[F]


def kernel(x, edge_index, edge_attr,
           W_ap, b_ap, W_ep, b_ep, W_msg, b_msg, g_msg, be_msg,
           W_l, W_r, att, b_gat, g_bn, be_bn,
           W_p1, b_p1, g_p, be_p, W_p2, b_p2):
    inputs = dict(x=x, edge_index=edge_index, edge_attr=edge_attr,
                  W_ap=W_ap, b_ap=b_ap, W_ep=W_ep, b_ep=b_ep, W_msg=W_msg,
                  b_msg=b_msg, g_msg=g_msg, be_msg=be_msg, W_l=W_l, W_r=W_r,
                  att=att, b_gat=b_gat, g_bn=g_bn, be_bn=be_bn, W_p1=W_p1,
                  b_p1=b_p1, g_p=g_p, be_p=be_p, W_p2=W_p2, b_p2=b_p2)
    F = max(10, _compute_F(edge_index))
    cfg, run = _get_runner(F)
    in_maps = _host_prep(inputs, cfg)
    results = run(in_maps)
    ys = []
    for k in range(N_CORES):
        yr = np.asarray(results[k]["y_raw"])
        ys.append(yr.T.ravel()[:NPC])
    y = np.concatenate(ys)
    return (y + float(np.asarray(b_p2).ravel()[0])).astype(np.float32)
